# revision 64
# baseline (speedup 1.0000x reference)
"""Trainium2 Bass kernel for LocalBackwardTemporalAttention.

Sharding: data-parallel over batch B=8 across the 8 NeuronCores (one batch
element per core, no collectives).

Optimized pipeline:
  - fp8 (e4m3) DoubleRow matmuls (2x PE throughput) for every GEMM except
    the attention score/AV matmuls and the final MLP: QKV projections,
    out_proj, and both mlpq GEMMs run fp8. Weights are absmax-quantized on
    the host with power-of-2 scales; activations are quantized raw (their
    errors are attenuated ~6x by the residual stream before reaching the
    output, validated numerically: rel err 0.0038 vs 0.0036 all-bf16).
    The final MLP stays bf16 because its quantization error would hit the
    output directly (measured 3.4-5% rel err in simulation).
  - dual-fp8 ldweights requires the k-subtile stride to be 16B-aligned,
    hence kv_fm's padded free dim (NKVP).
  - kv_fm / q / k_fm / o_fm / att / ln2t intermediates are SBUF-resident;
    region-precise tile deps let stages pipeline with no DRAM round trips.
  - attention in "S-transposed" form: S^T = k @ q^T puts the softmax axis
    on partitions; denominator via a PE ones-matmul (1/32-scaled so o_fm
    lands in fp8 normal range) with tile_position packing both head-halves
    into one 128-partition tile; one reciprocal + one DVE multiply writes
    o_fm in fp8 directly. 2-iteration software-pipeline stagger.
  - residual + res_ln + ln2 fused into the mlpq GEMM2 chunk loop, with the
    PE transposes deferred two iterations so the in-order PE queue never
    waits on the Vector LN chain.
  - LN transposes in stage A deferred one iteration (same reason); the
    k/v projection GEMMs and the q LN pass are interleaved into the LN
    loop.
  - final MLP g1/g2 interleaved: g2 token-chunks are emitted as soon as
    the g1 n-tiles covering their h1 columns are complete.
  - input-side DMA loads ride the SP HWDGE queue while e1/g1 output
    evacuations ride the Activation HWDGE queue, so next-stage prefetches
    never serialize behind the previous stage's store burst.
  - the actual inputs have every LayerNorm gain == 1 and every bias == 0;
    the host checks this and compiles the cheap specialization (the
    general fallback below keeps the original unoptimized bf16 pipeline).

Non-fp8 GEMMs run with bf16 inputs / f32 PSUM accumulation. LN/softmax
statistics in f32.
"""

import sys

sys.path.insert(0, "/opt/trn_rl_repo")

import numpy as np
import ml_dtypes

import concourse.bass as bass
import concourse.bacc as bacc_mod
import concourse.mybir as mybir
import concourse.tile as tile
from concourse.masks import make_identity

F32 = mybir.dt.float32
BF16 = mybir.dt.bfloat16
F8 = mybir.dt.float8e4
DR = mybir.MatmulPerfMode.DoubleRow
AF = mybir.ActivationFunctionType
ALU = mybir.AluOpType
AX = mybir.AxisListType

# problem shapes (hardcoded per spec)
B, HW, NF, E, M, H = 8, 16, 196, 1024, 4096, 16
T, D = HW - 1, E // H            # 15, 64
NKV, NQ = T * NF, NF             # 2940, 196
NTOK = HW * NF                   # 3136
EPS = 1e-6
P = 128
NT = 490                         # token n-tile for fm GEMMs (2940 = 6*490)
KE, KM = E // P, M // P          # 8, 32 k-chunks
ME, MM = E // P, M // P          # m-tiles
NKVP = 2944                      # kv_fm free-dim padded so the k-subtile
                                 # stride is 16B-aligned (dual-fp8 ldweights
                                 # ISA restriction)


def _ceil(a, b):
    return -(-a // b)


def build_nc(sc):
    scq, sck, scv, sco, sc1q, sc2q = (
        sc["wq"], sc["wk"], sc["wv"], sc["wo"], sc["w1q"], sc["w2q"])
    nc = bacc_mod.Bacc(None, target_bir_lowering=False)
    t = lambda n, s, d: nc.dram_tensor(n, s, d, kind="ExternalInput")

    x = t("x", [NTOK, E], BF16)
    wqT = t("wqT", [E, E], F8)
    wkT = t("wkT", [E, E], F8)
    wvT = t("wvT", [E, E], F8)
    woT = t("woT", [E, E], F8)
    w1qT = t("w1qT", [E, M], F8)
    w2qT = t("w2qT", [M, E], F8)
    w1T = t("w1T", [E, M], BF16)
    w2T = t("w2T", [M, E], BF16)
    out = nc.dram_tensor("out", [NKV, E], F32, kind="ExternalOutput")

    with tile.TileContext(nc) as tc:
        with tc.tile_pool(name="dram", bufs=1, space="DRAM") as dram, \
             tc.tile_pool(name="consts", bufs=1) as consts:
            # DRAM intermediates (too big / too late-lived for SBUF)
            kv_tm = dram.tile([NKV, E], BF16)
            v_tm = dram.tile([NKV, E], BF16)
            h1q_fm = dram.tile([M, NKV], F8)
            h1_fm = dram.tile([M, NKV], BF16)

            ident = consts.tile([P, P], BF16)
            make_identity(nc, ident)
            epst = consts.tile([P, 1], F32)
            nc.vector.memset(epst, EPS)
            # 1/32 so the denominator matmul yields den/32 and the
            # reciprocal broadcast gives 32/den: o_fm then carries 32*o,
            # placing fp8 o values in the normal-number range.
            ones = consts.tile([P, D], BF16)
            nc.vector.memset(ones, 1.0 / 32.0)

            # -------- stage A: LN (trivial gain/bias) + transpose --------
            def ln_pass(xin_rows, n_rows, fm_out, tm_out, fm_dt=F8):
                with tc.tile_pool(name="ln", bufs=3) as pool, \
                     tc.tile_pool(name="lnst", bufs=4) as stp, \
                     tc.tile_pool(name="lnps", bufs=4, space="PSUM") as psp:
                    for it in range(_ceil(n_rows, P)):
                        r0 = it * P
                        p = min(P, n_rows - r0)
                        xt = pool.tile([P, E], F32, name="ln_x")
                        nc.sync.dma_start(
                            out=xt[:p], in_=xin_rows[r0:r0 + p, :])
                        x3 = xt.rearrange("p (n f) -> p n f", n=2)
                        st = stp.tile([P, 2, 6], F32, name="ln_st")
                        for i in range(2):
                            nc.vector.bn_stats(out=st[:p, i, :], in_=x3[:p, i, :])
                        mv = stp.tile([P, 2], F32, name="ln_mv")
                        nc.vector.bn_aggr(out=mv[:p], in_=st[:p])
                        rs = stp.tile([P, 1], F32, name="ln_rs")
                        nc.scalar.activation(out=rs[:p], in_=mv[:p, 1:2],
                                             func=AF.Sqrt, bias=epst[:p])
                        nc.vector.reciprocal(out=rs[:p], in_=rs[:p])
                        y = pool.tile([P, E], BF16, name="ln_y")
                        nc.vector.tensor_scalar(
                            out=y[:p], in0=xt[:p], scalar1=mv[:p, 0:1],
                            scalar2=rs[:p], op0=ALU.subtract, op1=ALU.mult)
                        if tm_out is not None:
                            nc.sync.dma_start(out=tm_out[r0:r0 + p, :],
                                              in_=y[:p])
                        tp = psp.tile([P, KE, P], BF16, name="ln_tp")
                        for e in range(KE):
                            nc.tensor.transpose(
                                out=tp[:, e, :p],
                                in_=y[:p, e * P:(e + 1) * P],
                                identity=ident[:p, :p])
                        fmt = pool.tile([P, KE, P], fm_dt, name="ln_fmt")
                        nc.scalar.copy(out=fmt[:, :, :p], in_=tp[:, :, :p])
                        nc.sync.dma_start(out=fm_out[:, :, r0:r0 + p],
                                          in_=fmt[:, :, :p])

            # -------- fm GEMM: out[mo*P, n] = act(wT.T @ x_fm) --------
            def load_w(pool, wT, kc, ncols, tag, w_dt=BF16):
                wsb = pool.tile([P, kc, ncols], w_dt, name=tag + "_w")
                for k in range(kc):
                    nc.sync.dma_start(
                        out=wsb[:, k, :],
                        in_=wT[k * P:(k + 1) * P, :])
                return wsb

            def mm_loop(ps, lt, rt, kc, w, fp8):
                # ps f32 accumulate over kc k-chunks; lt/rt = (tile, slice)
                # for lhsT/rhs. fp8 pairs k-chunks into DoubleRow
                # (double-pumped) matmuls.
                if fp8:
                    for k in range(0, kc, 2):
                        nc.tensor.matmul(
                            ps, lhsT=lt[0][:, k:k + 2, lt[1]],
                            rhs=rt[0][:, k:k + 2, rt[1]],
                            start=(k == 0), stop=(k == kc - 2),
                            perf_mode=DR)
                else:
                    for k in range(kc):
                        nc.tensor.matmul(
                            ps, lhsT=lt[0][:, k, lt[1]],
                            rhs=rt[0][:, k, rt[1]],
                            start=(k == 0), stop=(k == kc - 1))

            def gemm_fm(wT, src, n_total, kc, mo, out_fm, act, tagp,
                        src_sbuf, out_sbuf, wsb_ext=None, fp8=False,
                        scale=1.0, ev_dt=BF16, x_dt=BF16, w_dt=BF16,
                        out_dma=None, vec_evac=False):
                ev_dma = out_dma if out_dma is not None else nc.sync
                with tc.tile_pool(name=tagp + "w", bufs=1) as wp, \
                     tc.tile_pool(name=tagp + "x", bufs=3) as xp, \
                     tc.tile_pool(name=tagp + "o", bufs=4) as op, \
                     tc.tile_pool(name=tagp + "ps", bufs=4, space="PSUM") as pp:
                    wsb = (wsb_ext if wsb_ext is not None
                           else load_w(wp, wT, kc, mo * P, tagp, w_dt))
                    nts = _ceil(n_total, NT)
                    for n in range(nts):
                        n0 = n * NT
                        w = min(NT, n_total - n0)
                        if src_sbuf:
                            xt, xoff = src, n0
                        else:
                            xt = xp.tile([P, kc, NT], x_dt, name=tagp + "_x")
                            nc.sync.dma_start(
                                out=xt[:, :, :w],
                                in_=src[:, n0:n0 + w].rearrange(
                                    "(k r) c -> r k c", r=P))
                            xoff = 0
                        for m in range(mo):
                            ps = pp.tile([P, NT], F32, name=tagp + "_ps")
                            mm_loop(ps[:, :w],
                                    (wsb, slice(m * P, (m + 1) * P)),
                                    (xt, slice(xoff, xoff + w)),
                                    kc, w, fp8)
                            if out_sbuf:
                                if vec_evac:
                                    nc.vector.tensor_scalar_mul(
                                        out_fm[:, m, n0:n0 + w],
                                        in0=ps[:, :w], scalar1=scale)
                                else:
                                    nc.scalar.activation(
                                        out=out_fm[:, m, n0:n0 + w],
                                        in_=ps[:, :w], func=act, scale=scale)
                            else:
                                ev = op.tile([P, NT], ev_dt, name=tagp + "_ev")
                                nc.scalar.activation(
                                    out=ev[:, :w], in_=ps[:, :w], func=act,
                                    scale=scale)
                                ev_dma.dma_start(
                                    out=out_fm[m * P:(m + 1) * P, n0:n0 + w],
                                    in_=ev[:, :w])

            # token-major GEMM: out_tm[c0:c1, :] = lhs_fm[:, c0:c1].T @ rhsT
            def gemm_tm(lhs_fm, kc, rhsT, n_out, out_tm, out_dt, tagp,
                        lhs_sbuf, post=None, wsb_ext=None, fp8=False,
                        lh_dt=BF16, w_dt=BF16, in_dma=None):
                nb = n_out // 512
                dma_eng = in_dma if in_dma is not None else nc.sync
                pend = []
                with tc.tile_pool(name=tagp + "w", bufs=1) as wp, \
                     tc.tile_pool(name=tagp + "h", bufs=3) as hp, \
                     tc.tile_pool(name=tagp + "o", bufs=4) as op, \
                     tc.tile_pool(name=tagp + "ps", bufs=4, space="PSUM") as pp:
                    wsb = (wsb_ext if wsb_ext is not None
                           else load_w(wp, rhsT, kc, n_out, tagp, w_dt))
                    for c in range(_ceil(NKV, P)):
                        c0 = c * P
                        p = min(P, NKV - c0)
                        if lhs_sbuf:
                            ht, hoff = lhs_fm, c0
                        else:
                            ht = hp.tile([P, kc, P], lh_dt, name=tagp + "_h")
                            dma_eng.dma_start(
                                out=ht[:, :, :p],
                                in_=lhs_fm[:, c0:c0 + p].rearrange(
                                    "(k r) c -> r k c", r=P))
                            hoff = 0
                        pss = []
                        for j in range(nb):
                            ps = pp.tile([P, 512], F32, name=tagp + "_ps")
                            mm_loop(ps[:p, :],
                                    (ht, slice(hoff, hoff + p)),
                                    (wsb, slice(j * 512, (j + 1) * 512)),
                                    kc, 512, fp8)
                            pss.append(ps)
                        if post is not None:
                            fin = post(c0, p, pss)
                            # run the deferred (PE-occupying) tail two
                            # iterations later, so the in-order PE queue
                            # never waits on the vector/scalar chain and
                            # the vector backlog has time to drain.
                            if len(pend) >= 2:
                                pend.pop(0)()
                            pend.append(fin)
                        else:
                            for j, ps in enumerate(pss):
                                ev = op.tile([P, 512], out_dt,
                                             name=tagp + "_ev")
                                nc.scalar.copy(out=ev[:p], in_=ps[:p, :])
                                nc.sync.dma_start(
                                    out=out_tm[c0:c0 + p,
                                               j * 512:(j + 1) * 512],
                                    in_=ev[:p])
                    while pend:
                        pend.pop(0)()

            # ================= stages A-D (SBUF-resident flow) ============
            # att_sb outlives persA/persB (feeds e1); ln2t_sb is allocated
            # after e1 into the space persA/persB freed (feeds g1). Both
            # SBUF-resident -> no DRAM round trip at stage boundaries.
            persX_cm = tc.tile_pool(name="persX", bufs=1)
            persX = persX_cm.__enter__()
            att_sb = persX.tile([P, KE, NKV], F8, name="att_sb")

            with tc.tile_pool(name="persA", bufs=1) as persA:
                kv_fm = persA.tile([P, KE, NKVP], F8, name="kv_fm")
                q_ln = persA.tile([P, KE, NQ], F8, name="q_ln")

                with tc.tile_pool(name="persB", bufs=1) as persB:
                    k_fm = persB.tile([P, KE, NKV], BF16, name="k_fm")
                    q_pr = persB.tile([P, KE, NQ], BF16, name="q_pr")
                    o_fm = persB.tile([P, KE, NKV], F8, name="o_fm")

                    # interleaved A (LN) + B (K/V gemms) emission: the
                    # in-order PE queue alternates LN transposes with GEMM
                    # tiles instead of queueing all transposes first.
                    with tc.tile_pool(name="ln", bufs=4) as lpool, \
                         tc.tile_pool(name="lnst", bufs=4) as lstp, \
                         tc.tile_pool(name="lnps", bufs=2, space="PSUM") as lpsp, \
                         tc.tile_pool(name="kpw", bufs=1) as kwp, \
                         tc.tile_pool(name="vpw", bufs=1) as vwp, \
                         tc.tile_pool(name="vpo", bufs=4) as vop, \
                         tc.tile_pool(name="kps", bufs=3, space="PSUM") as kpp, \
                         tc.tile_pool(name="vps", bufs=3, space="PSUM") as vpp:
                        wk_sb = kwp.tile([P, KE, E], F8, name="kp_w")
                        wv_sb = vwp.tile([P, KE, E], F8, name="vp_w")
                        for k in range(KE):
                            nc.sync.dma_start(
                                out=wk_sb[:, k, :],
                                in_=wkT[k * P:(k + 1) * P, :])
                            nc.sync.dma_start(
                                out=wv_sb[:, k, :],
                                in_=wvT[k * P:(k + 1) * P, :])

                        ys = {}

                        def a_ln(it, q=False):
                            r0 = it * P
                            p = min(P, (NQ if q else NKV) - r0)
                            xt = lpool.tile([P, E], BF16, name="ln_x")
                            nc.sync.dma_start(
                                out=xt[:p], in_=x[(NKV if q else 0) + r0:
                                                  (NKV if q else 0) + r0 + p, :])
                            x3 = xt.rearrange("p (n f) -> p n f", n=2)
                            st = lstp.tile([P, 2, 6], F32, name="ln_st")
                            for i in range(2):
                                nc.vector.bn_stats(out=st[:p, i, :],
                                                   in_=x3[:p, i, :])
                            mv = lstp.tile([P, 2], F32, name="ln_mv")
                            nc.vector.bn_aggr(out=mv[:p], in_=st[:p])
                            rs = lstp.tile([P, 1], F32, name="ln_rs")
                            nc.scalar.activation(out=rs[:p], in_=mv[:p, 1:2],
                                                 func=AF.Sqrt, bias=epst[:p])
                            nc.vector.reciprocal(out=rs[:p], in_=rs[:p])
                            y = lpool.tile([P, E], BF16, name="ln_y")
                            nc.vector.tensor_scalar(
                                out=y[:p], in0=xt[:p], scalar1=mv[:p, 0:1],
                                scalar2=rs[:p], op0=ALU.subtract, op1=ALU.mult)
                            if not q:
                                nc.sync.dma_start(out=kv_tm[r0:r0 + p, :],
                                                  in_=y[:p])
                            ys[(it, q)] = (y, p)

                        def a_fin(it, q=False):
                            # deferred PE transposes: emitted one iteration
                            # late so the PE queue never waits on the LN
                            # vector chain of the same iteration.
                            r0 = it * P
                            y, p = ys.pop((it, q))
                            tp = lpsp.tile([P, KE, P], BF16, name="ln_tp")
                            for e in range(KE):
                                nc.tensor.transpose(
                                    out=tp[:, e, :p],
                                    in_=y[:p, e * P:(e + 1) * P],
                                    identity=ident[:p, :p])
                            fmt = lpool.tile([P, KE, P], F8, name="ln_fmt")
                            nc.scalar.copy(out=fmt[:, :, :p], in_=tp[:, :, :p])
                            nc.sync.dma_start(
                                out=(q_ln if q else kv_fm)[:, :, r0:r0 + p],
                                in_=fmt[:, :, :p])

                        def v_chunk(cdx):
                            c0 = cdx * P
                            p = min(P, NKV - c0)
                            for j in range(2):
                                ps = vpp.tile([P, 512], F32, name="vp_ps")
                                mm_loop(ps[:p, :],
                                        (kv_fm, slice(c0, c0 + p)),
                                        (wv_sb, slice(j * 512, (j + 1) * 512)),
                                        KE, 512, True)
                                ev = vop.tile([P, 512], BF16, name="vp_ev")
                                nc.vector.tensor_scalar_mul(
                                    ev[:p], in0=ps[:p, :],
                                    scalar1=1.0 / scv)
                                nc.sync.dma_start(
                                    out=v_tm[c0:c0 + p,
                                             j * 512:(j + 1) * 512],
                                    in_=ev[:p])

                        def k_tile(n, n0=None, w=None):
                            if n0 is None:
                                n0 = n * NT
                                w = min(NT, NKV - n0)
                            for m in range(ME):
                                ps = kpp.tile([P, NT], F32, name="kp_ps")
                                mm_loop(ps[:, :w],
                                        (wk_sb, slice(m * P, (m + 1) * P)),
                                        (kv_fm, slice(n0, n0 + w)),
                                        KE, w, True)
                                nc.scalar.activation(
                                    out=k_fm[:, m, n0:n0 + w],
                                    in_=ps[:, :w], func=AF.Identity,
                                    scale=1.0 / sck)

                        # k/v GEMM pieces lag the LN writer by >=1 chunk so
                        # the in-order PE queue never waits on the LN ->
                        # fmt -> kv_fm DMA chain of the same iteration.
                        nkt = _ceil(NKV, NT)
                        nit = _ceil(NKV, P)
                        for it in range(nit):
                            a_ln(it)
                            if it == 8 or it == 12:
                                a_ln((it - 8) // 4, q=True)
                            if it >= 3:
                                v_chunk(it - 3)
                            if 2 <= it <= 4:
                                k_tile(0, n0=(it - 2) * P, w=P)
                            elif it == 5:
                                k_tile(0, n0=3 * P, w=NT - 3 * P)
                            elif it > 5:
                                nt = (it - 5) // 4
                                if (it - 5) % 4 == 0 and nt < nkt:
                                    k_tile(nt)
                            if it >= 1:
                                a_fin(it - 1)
                            if it == 9 or it == 13:
                                a_fin((it - 9) // 4, q=True)
                        a_fin(nit - 1)
                        for c in (nit - 3, nit - 2, nit - 1):
                            v_chunk(c)
                        for nt in range((nit - 5 + 3) // 4, nkt):
                            k_tile(nt)

                    gemm_fm(wqT[:, :], q_ln, NQ, KE, ME, q_pr,
                            AF.Identity, "qp", True, True, fp8=True,
                            scale=1.0 / scq, w_dt=F8)

                    # ---- stage C: attention (S^T form, 2-stage pipeline) --
                    with tc.tile_pool(name="cv", bufs=4) as cvp, \
                         tc.tile_pool(name="ce", bufs=6) as cep, \
                         tc.tile_pool(name="cr", bufs=3) as crp, \
                         tc.tile_pool(name="cps", bufs=3, space="PSUM") as cps, \
                         tc.tile_pool(name="cdn", bufs=1, space="PSUM") as cdn, \
                         tc.tile_pool(name="cwk", bufs=1, space="PSUM") as cwk:
                        mch = [(0, P), (P, NF - P)]      # m chunks: 128 + 68
                        niter = T * (H // 2)
                        st_ex = {}

                        def stage_a(i):
                            t_, hp_ = divmod(i, H // 2)
                            t0 = t_ * NF
                            vsb = cvp.tile([P, 2, P], BF16, name="c_v")
                            for jm, (m0, mj) in enumerate(mch):
                                nc.sync.dma_start(
                                    out=vsb[:mj, jm, :],
                                    in_=v_tm[t0 + m0:t0 + m0 + mj,
                                             hp_ * P:(hp_ + 1) * P])
                            ex = []
                            for jm, (m0, mj) in enumerate(mch):
                                # bufs=3: lets the next iteration's S matmuls
                                # start while this iteration's exp drains on
                                # the scalar engine.
                                ps = cps.tile([P, 2, 512], F32, name="c_ps")
                                et = cep.tile([P, 2, NQ], BF16, name="c_et")
                                for pi in range(2):
                                    d0 = pi * D
                                    nc.tensor.matmul(
                                        ps[:mj, pi, :NQ],
                                        lhsT=k_fm[d0:d0 + D, hp_,
                                                  t0 + m0:t0 + m0 + mj],
                                        rhs=q_pr[d0:d0 + D, hp_, :],
                                        start=True, stop=True)
                                nc.scalar.activation(
                                    out=et[:mj], in_=ps[:mj, :, :NQ],
                                    func=AF.Exp)
                                ex.append(et)
                            st_ex[i] = (vsb, ex)

                        def stage_c(i):
                            t_, hp_ = divmod(i, H // 2)
                            t0 = t_ * NF
                            vsb, ex = st_ex.pop(i)
                            # dn rows 0-63 = den(head pi=0)/32, rows 64-127 =
                            # den(pi=1)/32 (ones = 1/32); tile_position puts
                            # the pi=1 sums at partition base 64.
                            dn = cdn.tile([P, 256], F32, name="c_dn")
                            for pi in range(2):
                                for jm, (m0, mj) in enumerate(mch):
                                    nc.tensor.matmul(
                                        dn[pi * D:(pi + 1) * D, :NQ],
                                        lhsT=ones[:mj, :],
                                        rhs=ex[jm][:mj, pi, :],
                                        start=(jm == 0), stop=(jm == 1))
                            rb = crp.tile([P, NQ], F32, name="c_rb")
                            nc.vector.reciprocal_approx_fast(
                                out=rb, in_=dn[:, :NQ])
                            # AV into the o_fm partition layout: rows
                            # pi*64+d directly (e-index within chunk hp_)
                            wk_ = cwk.tile([P, 512], F32, name="c_wk")
                            for pi in range(2):
                                for jm, (m0, mj) in enumerate(mch):
                                    nc.tensor.matmul(
                                        wk_[pi * D:(pi + 1) * D, :NQ],
                                        lhsT=vsb[:mj, jm,
                                                 pi * D:(pi + 1) * D],
                                        rhs=ex[jm][:mj, pi, :],
                                        start=(jm == 0), stop=(jm == 1))
                            nc.vector.tensor_mul(
                                o_fm[:, hp_, t0:t0 + NF],
                                wk_[:, :NQ], rb)

                        import os as _os
                        if _os.environ.get("ATT_STUB"):
                            nc.vector.memset(o_fm, 0.001)
                        else:
                            for i in range(niter + 2):
                                if i < niter:
                                    stage_a(i)
                                if i >= 2:
                                    stage_c(i - 2)

                    # ---- stage D: out_proj ----
                    # o_fm carries 32*o; emit att_sb = fp8(64*att).
                    gemm_fm(woT[:, :], o_fm, NKV, KE, ME, att_sb,
                            AF.Identity, "op", True, True, fp8=True,
                            scale=2.0 / sco, w_dt=F8, vec_evac=True)

            # ================= stage E: mlpq, fused res_ln+ln2 ============
            gemm_fm(w1qT[:, :], att_sb, NKV, KE, MM, h1q_fm, AF.Gelu, "e1",
                    True, False, fp8=True, scale=1.0 / (64.0 * sc1q),
                    ev_dt=F8, w_dt=F8, out_dma=nc.scalar)
            persX_cm.__exit__(None, None, None)

            persY_cm = tc.tile_pool(name="persY", bufs=1)
            persY = persY_cm.__enter__()
            ln2t_sb = persY.tile([P, KE, NKV], BF16, name="ln2t_sb")

            from contextlib import ExitStack
            _g1st = ExitStack()
            _g1wp = _g1st.enter_context(tc.tile_pool(name="g1w", bufs=1))
            g1_wsb = load_w(_g1wp, w1T[:, :], KE, MM * P, "g1")

            with tc.tile_pool(name="f", bufs=3) as fp, \
                 tc.tile_pool(name="fst", bufs=4) as fst, \
                 tc.tile_pool(name="fr", bufs=3) as frp, \
                 tc.tile_pool(name="fps", bufs=2, space="PSUM") as fps:
                def post_e2(c0, p, pss):
                    rt = frp.tile([P, E], BF16, name="f_rt")
                    nc.sync.dma_start(out=rt[:p], in_=kv_tm[c0:c0 + p, :])
                    ev = fp.tile([P, E], F32, name="f_ev")
                    # res_ln statistics ride the scalar engine: the dequant
                    # accum gives the row sum (the kv_ln residual is
                    # mean-zero so it doesn't shift the mean), and one
                    # Square pass gives the sum of squares -> keeps the
                    # busy Vector engine out of the first LN's stats.
                    sm = fst.tile([P, 2], F32, name="f_sm")
                    for j, ps in enumerate(pss):
                        nc.scalar.activation(
                            out=ev[:p, j * 512:(j + 1) * 512], in_=ps[:p, :],
                            func=AF.Identity, scale=1.0 / sc2q,
                            accum_out=sm[:p, j:j + 1])
                    nc.vector.tensor_add(ev[:p], ev[:p], rt[:p])
                    sq = fp.tile([P, E], BF16, name="f_sq")
                    sq2 = fst.tile([P, 1], F32, name="f_sq2")
                    nc.scalar.activation(out=sq[:p], in_=ev[:p],
                                         func=AF.Square,
                                         accum_out=sq2[:p])
                    mv0 = fst.tile([P, 2], F32, name="f_mv0")
                    nc.vector.tensor_add(mv0[:p, 0:1], sm[:p, 0:1],
                                         sm[:p, 1:2])
                    nc.vector.tensor_scalar_mul(mv0[:p, 0:1],
                                                in0=mv0[:p, 0:1],
                                                scalar1=1.0 / E)
                    msq = fst.tile([P, 1], F32, name="f_msq")
                    nc.vector.tensor_mul(msq[:p], mv0[:p, 0:1],
                                         mv0[:p, 0:1])
                    nc.vector.tensor_scalar_mul(mv0[:p, 1:2], in0=sq2[:p],
                                                scalar1=1.0 / E)
                    nc.vector.tensor_sub(mv0[:p, 1:2], mv0[:p, 1:2],
                                         msq[:p])
                    cur = ev
                    for li in range(2):
                        if li == 0:
                            mv = mv0
                        else:
                            x3 = cur.rearrange("p (n f) -> p n f", n=2)
                            st = fst.tile([P, 2, 6], F32, name="f_st")
                            for i in range(2):
                                nc.vector.bn_stats(out=st[:p, i, :],
                                                   in_=x3[:p, i, :])
                            mv = fst.tile([P, 2], F32, name="f_mv")
                            nc.vector.bn_aggr(out=mv[:p], in_=st[:p])
                        rs = fst.tile([P, 1], F32, name="f_rs")
                        nc.scalar.activation(out=rs[:p], in_=mv[:p, 1:2],
                                             func=AF.Sqrt, bias=epst[:p])
                        nc.vector.reciprocal(out=rs[:p], in_=rs[:p])
                        y = fp.tile([P, E], F32 if li == 0 else BF16,
                                    name=f"f_y{li}")
                        nc.vector.tensor_scalar(
                            out=y[:p], in0=cur[:p], scalar1=mv[:p, 0:1],
                            scalar2=rs[:p], op0=ALU.subtract, op1=ALU.mult)
                        cur = y

                    def fin():
                        tp = fps.tile([P, KE, P], BF16, name="f_tp")
                        for e in range(KE):
                            nc.tensor.transpose(
                                out=tp[:, e, :p],
                                in_=cur[:p, e * P:(e + 1) * P],
                                identity=ident[:p, :p])
                        fmt = fp.tile([P, KE, P], BF16, name="f_fmt")
                        nc.scalar.copy(out=fmt[:, :, :p], in_=tp[:, :, :p])
                        nc.sync.dma_start(
                            out=ln2t_sb[:, :, c0:c0 + p],
                            in_=fmt[:, :, :p])
                    return fin

                gemm_tm(h1q_fm, KM, w2qT[:, :], E, None, F32, "e2",
                        False, post=post_e2, fp8=True, lh_dt=F8, w_dt=F8)

            # ================= stage G: mlp (g1/g2 interleaved) ============
            # g2 c-chunks are emitted as soon as the g1 n-tiles covering
            # their h1 columns are done, so the PE never drains at the
            # g1 -> g2 boundary.
            with tc.tile_pool(name="g2w", bufs=1) as _g2wp, \
                 tc.tile_pool(name="g1o", bufs=4) as g1op, \
                 tc.tile_pool(name="g1ps", bufs=4, space="PSUM") as g1pp, \
                 tc.tile_pool(name="g2h", bufs=2) as g2hp, \
                 tc.tile_pool(name="g2o", bufs=3) as g2op, \
                 tc.tile_pool(name="g2ps", bufs=4, space="PSUM") as g2pp:
                g2_wsb = load_w(_g2wp, w2T[:, :], KM, E, "g2")

                def g1_ntile(n):
                    n0 = n * NT
                    w = min(NT, NKV - n0)
                    for m in range(MM):
                        ps = g1pp.tile([P, NT], F32, name="g1_ps")
                        mm_loop(ps[:, :w],
                                (g1_wsb, slice(m * P, (m + 1) * P)),
                                (ln2t_sb, slice(n0, n0 + w)),
                                KE, w, False)
                        ev = g1op.tile([P, NT], BF16, name="g1_ev")
                        nc.scalar.activation(
                            out=ev[:, :w], in_=ps[:, :w], func=AF.Gelu)
                        nc.scalar.dma_start(
                            out=h1_fm[m * P:(m + 1) * P, n0:n0 + w],
                            in_=ev[:, :w])

                def g2_chunk(c):
                    c0 = c * P
                    p = min(P, NKV - c0)
                    ht = g2hp.tile([P, KM, P], BF16, name="g2_h")
                    nc.sync.dma_start(
                        out=ht[:, :, :p],
                        in_=h1_fm[:, c0:c0 + p].rearrange(
                            "(k r) c -> r k c", r=P))
                    for j in range(2):
                        ps = g2pp.tile([P, 512], F32, name="g2_ps")
                        mm_loop(ps[:p, :],
                                (ht, slice(0, p)),
                                (g2_wsb, slice(j * 512, (j + 1) * 512)),
                                KM, 512, False)
                        ev = g2op.tile([P, 512], F32, name="g2_ev")
                        nc.scalar.copy(out=ev[:p], in_=ps[:p, :])
                        nc.sync.dma_start(
                            out=out[c0:c0 + p, j * 512:(j + 1) * 512],
                            in_=ev[:p])

                ngt = _ceil(NKV, NT)
                ncc = _ceil(NKV, P)
                done = 0
                for n in range(ngt):
                    g1_ntile(n)
                    if n >= 1:
                        # chunks fully covered by n-tiles 0..n-1
                        avail = (n * NT) // P
                        for c in range(done, min(avail, ncc)):
                            g2_chunk(c)
                        done = min(avail, ncc)
                for c in range(done, ncc):
                    g2_chunk(c)
            _g1st.close()
            persY_cm.__exit__(None, None, None)

    nc.compile()
    return nc


# ====================================================================
# general fallback (original pipeline, used only if the inputs have
# non-trivial LN gains/biases or linear biases)
# ====================================================================

def _bcast_ap(handle, n):
    a = handle[:]
    return bass.AP(tensor=a.tensor, offset=a.offset, ap=[[0, n], list(a.ap[0])])


def _col_ap(handle, mo):
    a = handle[:]
    return bass.AP(tensor=a.tensor, offset=a.offset, ap=[[1, P], [P, mo]])


def build_nc_general():
    nc = bacc_mod.Bacc(None, target_bir_lowering=False)
    t = lambda n, s, d: nc.dram_tensor(n, s, d, kind="ExternalInput")

    x = t("x", [NTOK, E], F32)
    wqT = t("wqT", [E, E], BF16)
    wkT = t("wkT", [E, E], BF16)
    wvT = t("wvT", [E, E], BF16)
    woT = t("woT", [E, E], BF16)
    w1qT = t("w1qT", [E, M], BF16)
    w2qT = t("w2qT", [M, E], BF16)
    w1T = t("w1T", [E, M], BF16)
    w2T = t("w2T", [M, E], BF16)
    bqs = t("bqs", [E], F32)
    bk = t("bk", [E], F32)
    bv = t("bv", [E], F32)
    bo = t("bo", [E], F32)
    b1q = t("b1q", [M], F32)
    b2q = t("b2q", [E], F32)
    b1 = t("b1", [M], F32)
    b2 = t("b2", [E], F32)
    gq = t("gq", [E], F32)
    bbq = t("bbq", [E], F32)
    gkv = t("gkv", [E], F32)
    bbkv = t("bbkv", [E], F32)
    gres = t("gres", [E], F32)
    bbres = t("bbres", [E], F32)
    gln2 = t("gln2", [E], F32)
    bbln2 = t("bbln2", [E], F32)
    out = nc.dram_tensor("out", [NKV, E], F32, kind="ExternalOutput")

    with tile.TileContext(nc) as tc:
        with tc.tile_pool(name="dram", bufs=1, space="DRAM") as dram, \
             tc.tile_pool(name="consts", bufs=1) as consts:
            kv_fm = dram.tile([E, NKV], BF16)
            kv_tm = dram.tile([NKV, E], BF16)
            q_fm = dram.tile([E, NQ], BF16)
            k_fm = dram.tile([E, NKV], BF16)
            v_tm = dram.tile([NKV, E], BF16)
            o_fm = dram.tile([E, NKV], BF16)
            att_fm = dram.tile([E, NKV], BF16)
            h1q_fm = dram.tile([M, NKV], BF16)
            qpre_tm = dram.tile([NKV, E], F32)
            ln2t_fm = dram.tile([E, NKV], BF16)
            h1_fm = dram.tile([M, NKV], BF16)

            ident = consts.tile([P, P], BF16)
            make_identity(nc, ident)
            epst = consts.tile([P, 1], F32)
            nc.vector.memset(epst, EPS)

            def ln_pass(xin_rows, gain, bias_, n_rows, fm_out, tm_out):
                with tc.tile_pool(name="ln", bufs=3) as pool, \
                     tc.tile_pool(name="lnst", bufs=4) as stp, \
                     tc.tile_pool(name="lnps", bufs=4, space="PSUM") as psp, \
                     tc.tile_pool(name="lng", bufs=1) as gp:
                    gt = gp.tile([P, E], F32, name="ln_gain")
                    bt = gp.tile([P, E], F32, name="ln_bias")
                    nc.sync.dma_start(out=gt, in_=_bcast_ap(gain, P))
                    nc.sync.dma_start(out=bt, in_=_bcast_ap(bias_, P))
                    for it in range(_ceil(n_rows, P)):
                        r0 = it * P
                        p = min(P, n_rows - r0)
                        xt = pool.tile([P, E], F32, name="ln_x")
                        nc.sync.dma_start(
                            out=xt[:p], in_=xin_rows[r0:r0 + p, :])
                        x3 = xt.rearrange("p (n f) -> p n f", n=2)
                        st = stp.tile([P, 2, 6], F32, name="ln_st")
                        for i in range(2):
                            nc.vector.bn_stats(out=st[:p, i, :], in_=x3[:p, i, :])
                        mv = stp.tile([P, 2], F32, name="ln_mv")
                        nc.vector.bn_aggr(out=mv[:p], in_=st[:p])
                        rs = stp.tile([P, 1], F32, name="ln_rs")
                        nc.scalar.activation(out=rs[:p], in_=mv[:p, 1:2],
                                             func=AF.Sqrt, bias=epst[:p])
                        nc.vector.reciprocal(out=rs[:p], in_=rs[:p])
                        y = pool.tile([P, E], BF16, name="ln_y")
                        nc.vector.tensor_scalar(
                            out=y[:p], in0=xt[:p], scalar1=mv[:p, 0:1],
                            scalar2=rs[:p], op0=ALU.subtract, op1=ALU.mult)
                        nc.vector.tensor_mul(y[:p], y[:p], gt[:p])
                        nc.vector.tensor_add(y[:p], y[:p], bt[:p])
                        if tm_out is not None:
                            nc.sync.dma_start(out=tm_out[r0:r0 + p, :], in_=y[:p])
                        tp = psp.tile([P, KE, P], BF16, name="ln_tp")
                        for e in range(KE):
                            nc.tensor.transpose(
                                out=tp[:, e, :p],
                                in_=y[:p, e * P:(e + 1) * P],
                                identity=ident[:p, :p])
                        fmt = pool.tile([P, KE, P], BF16, name="ln_fmt")
                        nc.scalar.copy(out=fmt, in_=tp)
                        dst = fm_out[:, r0:r0 + p].rearrange(
                            "(e r) c -> r e c", r=P)
                        nc.sync.dma_start(out=dst, in_=fmt[:, :, :p])

            ln_pass(x[:NKV, :], gkv, bbkv, NKV, kv_fm, kv_tm)
            ln_pass(x[NKV:, :], gq, bbq, NQ, q_fm, None)

            def gemm_fm(wT, x_fm_ap, n_total, kc, mo, bias_h, out_fm, act, tagp):
                with tc.tile_pool(name=tagp + "w", bufs=1) as wp, \
                     tc.tile_pool(name=tagp + "x", bufs=3) as xp, \
                     tc.tile_pool(name=tagp + "o", bufs=4) as op, \
                     tc.tile_pool(name=tagp + "ps", bufs=4, space="PSUM") as pp, \
                     tc.tile_pool(name=tagp + "b", bufs=1) as bp:
                    wsb = wp.tile([P, kc, mo * P], BF16, name=tagp + "_w")
                    for k in range(kc):
                        nc.sync.dma_start(
                            out=wsb[:, k, :],
                            in_=wT[k * P:(k + 1) * P, :])
                    bsb = bp.tile([P, mo], F32, name=tagp + "_b")
                    nc.sync.dma_start(out=bsb, in_=_col_ap(bias_h, mo))
                    nts = _ceil(n_total, NT)
                    for n in range(nts):
                        n0 = n * NT
                        w = min(NT, n_total - n0)
                        xt = xp.tile([P, kc, NT], BF16, name=tagp + "_x")
                        nc.sync.dma_start(
                            out=xt[:, :, :w],
                            in_=x_fm_ap[:, n0:n0 + w].rearrange(
                                "(k r) c -> r k c", r=P))
                        for m in range(mo):
                            ps = pp.tile([P, NT], F32, name=tagp + "_ps")
                            for k in range(kc):
                                nc.tensor.matmul(
                                    ps[:, :w],
                                    lhsT=wsb[:, k, m * P:(m + 1) * P],
                                    rhs=xt[:, k, :w],
                                    start=(k == 0), stop=(k == kc - 1))
                            ev = op.tile([P, NT], BF16, name=tagp + "_ev")
                            nc.scalar.activation(
                                out=ev[:, :w], in_=ps[:, :w], func=act,
                                bias=bsb[:, m:m + 1])
                            nc.sync.dma_start(
                                out=out_fm[m * P:(m + 1) * P, n0:n0 + w],
                                in_=ev[:, :w])

            gemm_fm(wkT[:, :], kv_fm, NKV, KE, ME, bk, k_fm, AF.Identity, "kp")
            gemm_fm(wqT[:, :], q_fm, NQ, KE, ME, bqs, q_fm, AF.Identity, "qp")

            def gemm_tm(lhs_fm, kc, rhsT, n_out, bias_free_h, out_tm, resid_tm,
                        out_dt, tagp):
                nb = n_out // 512
                with tc.tile_pool(name=tagp + "w", bufs=1) as wp, \
                     tc.tile_pool(name=tagp + "h", bufs=3) as hp, \
                     tc.tile_pool(name=tagp + "o", bufs=4) as op, \
                     tc.tile_pool(name=tagp + "r", bufs=3) as rp, \
                     tc.tile_pool(name=tagp + "ps", bufs=4, space="PSUM") as pp, \
                     tc.tile_pool(name=tagp + "b", bufs=1) as bp:
                    wsb = wp.tile([P, kc, n_out], BF16, name=tagp + "_w")
                    for k in range(kc):
                        nc.sync.dma_start(
                            out=wsb[:, k, :],
                            in_=rhsT[k * P:(k + 1) * P, :])
                    bsb = bp.tile([P, n_out], F32, name=tagp + "_b")
                    nc.sync.dma_start(out=bsb, in_=_bcast_ap(bias_free_h, P))
                    for c in range(_ceil(NKV, P)):
                        c0 = c * P
                        p = min(P, NKV - c0)
                        ht = hp.tile([P, kc, P], BF16, name=tagp + "_h")
                        nc.sync.dma_start(
                            out=ht[:, :, :p],
                            in_=lhs_fm[:, c0:c0 + p].rearrange(
                                "(k r) c -> r k c", r=P))
                        rt = None
                        if resid_tm is not None:
                            rt = rp.tile([P, n_out], BF16, name=tagp + "_r")
                            nc.sync.dma_start(out=rt[:p],
                                              in_=resid_tm[c0:c0 + p, :])
                        for j in range(nb):
                            ps = pp.tile([P, 512], F32, name=tagp + "_ps")
                            for k in range(kc):
                                nc.tensor.matmul(
                                    ps[:p, :],
                                    lhsT=ht[:, k, :p],
                                    rhs=wsb[:, k, j * 512:(j + 1) * 512],
                                    start=(k == 0), stop=(k == kc - 1))
                            ev = op.tile([P, 512], out_dt, name=tagp + "_ev")
                            nc.vector.tensor_add(
                                ev[:p], ps[:p, :], bsb[:p, j * 512:(j + 1) * 512])
                            if rt is not None:
                                nc.vector.tensor_add(
                                    ev[:p], ev[:p], rt[:p, j * 512:(j + 1) * 512])
                            nc.sync.dma_start(
                                out=out_tm[c0:c0 + p, j * 512:(j + 1) * 512],
                                in_=ev[:p])

            gemm_tm(kv_fm, KE, wvT[:, :], E, bv, v_tm, None, BF16, "vp")

            with tc.tile_pool(name="cq", bufs=1) as cqp, \
                 tc.tile_pool(name="ckv", bufs=4) as ckv, \
                 tc.tile_pool(name="cat", bufs=3) as cat, \
                 tc.tile_pool(name="cst", bufs=4) as cst, \
                 tc.tile_pool(name="co", bufs=3) as cop, \
                 tc.tile_pool(name="cps", bufs=2, space="PSUM") as cps, \
                 tc.tile_pool(name="cpt", bufs=2, space="PSUM") as cpt, \
                 tc.tile_pool(name="cpo", bufs=2, space="PSUM") as cpo:
                qsb = cqp.tile([P, KE, NQ], BF16)
                nc.sync.dma_start(
                    out=qsb, in_=q_fm[:, :].rearrange("(e r) c -> r e c", r=P))
                nch = [(0, P), (P, NQ - P)]
                for t_ in range(T):
                    t0 = t_ * NF
                    osb = cop.tile([P, KE, NQ], BF16, name="c_osb")
                    for hp in range(H // 2):
                      ksb = ckv.tile([P, NQ], BF16, name="c_k")
                      nc.sync.dma_start(
                          out=ksb,
                          in_=k_fm[hp * P:(hp + 1) * P, t0:t0 + NF])
                      vsb = ckv.tile([P, 2, P], BF16, name="c_v")
                      for j, (m0, mj) in enumerate(nch):
                          nc.sync.dma_start(
                              out=vsb[:mj, j, :],
                              in_=v_tm[t0 + m0:t0 + m0 + mj,
                                       hp * P:(hp + 1) * P])
                      for pi in range(2):
                        d0 = pi * D
                        ps = cps.tile([P, 2, 512], F32, name="c_ps")
                        for j, (n0, pn) in enumerate(nch):
                            nc.tensor.matmul(
                                ps[:pn, j, :NQ],
                                lhsT=qsb[d0:d0 + D, hp, n0:n0 + pn],
                                rhs=ksb[d0:d0 + D, :],
                                start=True, stop=True)
                        nm = cst.tile([P, 2], F32, name="c_nm")
                        nc.vector.reduce_max(
                            out=nm, in_=ps[:, :, :NQ], axis=AX.X, negate=True)
                        asb = cat.tile([P, 2, NQ], BF16, name="c_asb")
                        sm = cst.tile([P, 2], F32, name="c_sm")
                        for j, (n0, pn) in enumerate(nch):
                            nc.scalar.activation(
                                out=asb[:pn, j, :], in_=ps[:pn, j, :NQ],
                                func=AF.Exp, bias=nm[:pn, j:j + 1],
                                accum_out=sm[:pn, j:j + 1])
                        rc = cst.tile([P, 2], F32, name="c_rc")
                        nc.vector.reciprocal(out=rc, in_=sm)
                        for j, (n0, pn) in enumerate(nch):
                            nc.vector.tensor_scalar_mul(
                                asb[:pn, j, :], in0=asb[:pn, j, :],
                                scalar1=rc[:pn, j:j + 1])
                        pt = cpt.tile([P, 2, NQ], BF16, name="c_pt")
                        for jn, (n0, pn) in enumerate(nch):
                            for jm, (m0, mj) in enumerate(nch):
                                nc.tensor.transpose(
                                    out=pt[:mj, jm, n0:n0 + pn],
                                    in_=asb[:pn, jn, m0:m0 + mj],
                                    identity=ident[:pn, :pn])
                        atT = cat.tile([P, 2, NQ], BF16, name="c_atT")
                        nc.scalar.copy(out=atT, in_=pt)
                        po = cpo.tile([D, NQ], F32, name="c_po")
                        for jm, (m0, mj) in enumerate(nch):
                            nc.tensor.matmul(
                                po[:, :],
                                lhsT=vsb[:mj, jm, d0:d0 + D],
                                rhs=atT[:mj, jm, :],
                                start=(jm == 0), stop=(jm == 1))
                        nc.scalar.copy(out=osb[d0:d0 + D, hp, :],
                                       in_=po)
                    nc.sync.dma_start(
                        out=o_fm[:, t0:t0 + NF].rearrange(
                            "(e r) c -> r e c", r=P),
                        in_=osb)

            gemm_fm(woT[:, :], o_fm, NKV, KE, ME, bo, att_fm, AF.Identity, "op")

            gemm_fm(w1qT[:, :], att_fm, NKV, KE, MM, b1q, h1q_fm, AF.Gelu, "e1")
            gemm_tm(h1q_fm, KM, w2qT[:, :], E, b2q, qpre_tm, kv_tm, F32, "e2")

            with tc.tile_pool(name="f", bufs=3) as fp, \
                 tc.tile_pool(name="fst", bufs=4) as fst, \
                 tc.tile_pool(name="fps", bufs=4, space="PSUM") as fps, \
                 tc.tile_pool(name="fg", bufs=1) as fg:
                g1 = fg.tile([P, E], F32)
                bb1 = fg.tile([P, E], F32)
                g2 = fg.tile([P, E], F32)
                bb2 = fg.tile([P, E], F32)
                nc.sync.dma_start(out=g1, in_=_bcast_ap(gres, P))
                nc.sync.dma_start(out=bb1, in_=_bcast_ap(bbres, P))
                nc.sync.dma_start(out=g2, in_=_bcast_ap(gln2, P))
                nc.sync.dma_start(out=bb2, in_=_bcast_ap(bbln2, P))
                for it in range(_ceil(NKV, P)):
                    r0 = it * P
                    p = min(P, NKV - r0)
                    xt = fp.tile([P, E], F32, name="f_x")
                    nc.sync.dma_start(out=xt[:p], in_=qpre_tm[r0:r0 + p, :])
                    cur = xt
                    for li, (gg, bb) in enumerate(((g1, bb1), (g2, bb2))):
                        x3 = cur.rearrange("p (n f) -> p n f", n=2)
                        st = fst.tile([P, 2, 6], F32, name="f_st")
                        for i in range(2):
                            nc.vector.bn_stats(out=st[:p, i, :], in_=x3[:p, i, :])
                        mv = fst.tile([P, 2], F32, name="f_mv")
                        nc.vector.bn_aggr(out=mv[:p], in_=st[:p])
                        rs = fst.tile([P, 1], F32, name="f_rs")
                        nc.scalar.activation(out=rs[:p], in_=mv[:p, 1:2],
                                             func=AF.Sqrt, bias=epst[:p])
                        nc.vector.reciprocal(out=rs[:p], in_=rs[:p])
                        y = fp.tile([P, E], F32 if li == 0 else BF16,
                                    name=f"f_y{li}")
                        nc.vector.tensor_scalar(
                            out=y[:p], in0=cur[:p], scalar1=mv[:p, 0:1],
                            scalar2=rs[:p], op0=ALU.subtract, op1=ALU.mult)
                        nc.vector.tensor_mul(y[:p], y[:p], gg[:p])
                        nc.vector.tensor_add(y[:p], y[:p], bb[:p])
                        cur = y
                    tp = fps.tile([P, KE, P], BF16, name="f_tp")
                    for e in range(KE):
                        nc.tensor.transpose(
                            out=tp[:, e, :p],
                            in_=cur[:p, e * P:(e + 1) * P],
                            identity=ident[:p, :p])
                    fmt = fp.tile([P, KE, P], BF16, name="f_fmt")
                    nc.scalar.copy(out=fmt, in_=tp)
                    nc.sync.dma_start(
                        out=ln2t_fm[:, r0:r0 + p].rearrange(
                            "(e r) c -> r e c", r=P),
                        in_=fmt[:, :, :p])

            gemm_fm(w1T[:, :], ln2t_fm, NKV, KE, MM, b1, h1_fm, AF.Gelu, "g1")
            gemm_tm(h1_fm, KM, w2T[:, :], E, b2, out[:, :], None, F32, "g2")

    nc.compile()
    return nc


_NC = {}


def _get_nc(triv, scales=None):
    key = (triv, tuple(sorted(scales.items())) if scales else None)
    if key not in _NC:
        _NC[key] = build_nc(scales) if triv else build_nc_general()
    return _NC[key]


def _check_trivial(inputs):
    f32 = lambda a: np.asarray(a, dtype=np.float32)
    zeros = ["in_proj_b", "out_proj_b", "mlpq_b1", "mlpq_b2", "mlp_b1",
             "mlp_b2", "ln_q_b", "ln_kv_b", "res_ln_b", "ln2_b"]
    ones_ = ["ln_q_g", "ln_kv_g", "res_ln_g", "ln2_g"]
    return (all(np.all(f32(inputs[k]) == 0.0) for k in zeros)
            and all(np.all(f32(inputs[k]) == 1.0) for k in ones_))


def _qw8(w):
    """Power-of-2 absmax scale into fp8 e4m3 normal range."""
    am = float(np.abs(w).max())
    s = 2.0 ** np.floor(np.log2(240.0 / am)) if am > 0 else 1.0
    q = np.ascontiguousarray(
        np.clip(w * s, -240.0, 240.0).astype(ml_dtypes.float8_e4m3))
    return q, float(s)


def _prep_shared_w(inputs, triv=True):
    f32 = lambda a: np.ascontiguousarray(np.asarray(a, dtype=np.float32))
    bf = lambda a: np.ascontiguousarray(
        np.asarray(a, dtype=np.float32).astype(ml_dtypes.bfloat16))
    ipw = f32(inputs["in_proj_w"])
    wq, wk, wv = ipw[:E], ipw[E:2 * E], ipw[2 * E:]
    s = 1.0 / np.sqrt(np.float32(D))
    if not triv:
        return {
            "wqT": bf(wq.T * s), "wkT": bf(wk.T), "wvT": bf(wv.T),
            "woT": bf(f32(inputs["out_proj_w"]).T),
            "w1qT": bf(f32(inputs["mlpq_w1"]).T),
            "w2qT": bf(f32(inputs["mlpq_w2"]).T),
            "w1T": bf(f32(inputs["mlp_w1"]).T),
            "w2T": bf(f32(inputs["mlp_w2"]).T),
        }, None
    wqT, scq = _qw8(wq.T * s)
    wkT, sck = _qw8(wk.T)
    wvT, scv = _qw8(wv.T)
    woT, sco = _qw8(f32(inputs["out_proj_w"]).T)
    w1qT, sc1q = _qw8(f32(inputs["mlpq_w1"]).T)
    w2qT, sc2q = _qw8(f32(inputs["mlpq_w2"]).T)
    scales = {"wq": scq, "wk": sck, "wv": scv, "wo": sco,
              "w1q": sc1q, "w2q": sc2q}
    return {
        "wqT": wqT, "wkT": wkT, "wvT": wvT, "woT": woT,
        "w1qT": w1qT, "w2qT": w2qT,
        "w1T": bf(f32(inputs["mlp_w1"]).T),
        "w2T": bf(f32(inputs["mlp_w2"]).T),
    }, scales


def _prep_in_maps(inputs, triv):
    f32 = lambda a: np.ascontiguousarray(np.asarray(a, dtype=np.float32))
    x = f32(inputs["inputs"])                       # (B,HW,NF,E)
    shared, scales = _prep_shared_w(inputs, triv)
    if not triv:
        ipb = f32(inputs["in_proj_b"])
        s = 1.0 / np.sqrt(np.float32(D))
        shared.update({
            "bqs": f32(ipb[:E] * s), "bk": f32(ipb[E:2 * E]),
            "bv": f32(ipb[2 * E:]),
            "bo": f32(inputs["out_proj_b"]),
            "b1q": f32(inputs["mlpq_b1"]), "b2q": f32(inputs["mlpq_b2"]),
            "b1": f32(inputs["mlp_b1"]), "b2": f32(inputs["mlp_b2"]),
            "gq": f32(inputs["ln_q_g"]), "bbq": f32(inputs["ln_q_b"]),
            "gkv": f32(inputs["ln_kv_g"]), "bbkv": f32(inputs["ln_kv_b"]),
            "gres": f32(inputs["res_ln_g"]), "bbres": f32(inputs["res_ln_b"]),
            "gln2": f32(inputs["ln2_g"]), "bbln2": f32(inputs["ln2_b"]),
        })
    if triv:
        x = x.astype(ml_dtypes.bfloat16)
    return [dict(shared, x=np.ascontiguousarray(x[b].reshape(NTOK, E)))
            for b in range(B)], scales


def _run(inputs, trace=False):
    from concourse.bass_utils import run_bass_kernel_spmd
    triv = _check_trivial(inputs)
    in_maps, scales = _prep_in_maps(inputs, triv)
    nc = _get_nc(triv, scales)
    res = run_bass_kernel_spmd(nc, in_maps, core_ids=list(range(B)),
                               trace=trace)
    outs = np.stack([r["out"].reshape(T, NF, E) for r in res.results])
    return outs, res


def kernel(**inputs) -> np.ndarray:
    outs, _ = _run(inputs, trace=False)
    return outs

